# revision 4
# baseline (speedup 1.0000x reference)
"""Sparse-attention kernel, query-parallel restructure for 8 trn2 cores.

Sharding: core c -> (b = c // 4, g = c % 4). Data-parallel over batch B=2.
Within a batch's 4 cores:
  - Phase A: k/v projection sharded by s-QUARTER (each core computes all 4
    kv heads for s in [512g, 512(g+1))), k-norm + RoPE applied locally.
  - kv AllGather (2 ops, head-pairs) inside the replica group -> every core
    holds full-S k/v for all 4 kv heads. Triggered per head-pair as phase A
    drains so it overlaps phase Q.
  - Phase Q: q projection for ALL 16 heads x 256 STRIDED queries
    (local j <-> global sorted query 4j+g; striding balances the causal
    load across cores).
  - Phase B: attention for 16 heads x 256 local queries. Scores in
    [s, q] orientation; 4 s-tiles packed flush at PSUM bank boundaries so
    one exp covers a whole pack; boundary masks on DVE; rowsum via
    ones-matmul; late 1/rowsum.
  - Phase C: o_proj fully LOCAL (full Wo, contraction over all 16 heads,
    output = this core's 256 query rows). No collective after o_proj --
    the tail ReduceScatter of the head-parallel design is gone.
Host pre-norms hidden states (f64), folds w_ln into Wq/Wk/Wv, folds
w_qn/w_kn + rotate-half signs into cos/sin factors, adds bo and scatters
the K rows back into [B,S,D].
"""

import numpy as np
import ml_dtypes
import concourse.bacc as bacc
import concourse.tile as tile
from concourse import mybir
from concourse.bass_utils import run_bass_kernel_spmd

B, S, K, D, H, HKV, HD = 2, 2048, 1024, 2048, 16, 4, 128
EPS = 1e-6
SCALE = float(HD) ** -0.5
NCORES = 8
NT = S // 128            # 16 s-tiles
NDC = D // 128           # 16 d-chunks
KL = K // 4              # 256 local queries per core
SQ = S // 4              # 512-wide s-quarter per core

F32 = mybir.dt.float32
BF16 = mybir.dt.bfloat16
AFT = mybir.ActivationFunctionType
BF = ml_dtypes.bfloat16

_BUILD_CACHE = {}
_LAST_IN_MAPS = None


def _build(klo_u, khi_max):
    nc = bacc.Bacc("TRN2", target_bir_lowering=False, debug=False,
                   num_devices=NCORES)

    mw = [max(0, khi_max[t] - klo_u[t]) for t in range(NT)]
    moff = np.concatenate([[0], np.cumsum(mw)]).astype(int)
    MW = int(moff[-1])

    p = {}
    p["hT"] = nc.declare_dram_parameter("hT", [128, NDC * SQ], BF16,
                                        isOutput=False)
    p["hq"] = nc.declare_dram_parameter("hq", [128, NDC * KL], BF16,
                                        isOutput=False)
    p["wq"] = nc.declare_dram_parameter("wq", [128, NDC * H * HD], BF16,
                                        isOutput=False)
    p["wk"] = nc.declare_dram_parameter("wk", [128, NDC * HKV * HD], BF16,
                                        isOutput=False)
    p["wv"] = nc.declare_dram_parameter("wv", [128, NDC * HKV * HD], BF16,
                                        isOutput=False)
    p["wo"] = nc.declare_dram_parameter("wo", [H * HD, D], BF16,
                                        isOutput=False)
    p["cosq"] = nc.declare_dram_parameter("cosq", [HD, KL], BF16,
                                          isOutput=False)
    p["sinq"] = nc.declare_dram_parameter("sinq", [HD, KL], BF16,
                                          isOutput=False)
    p["cosk"] = nc.declare_dram_parameter("cosk", [HD, SQ], BF16,
                                          isOutput=False)
    p["sink"] = nc.declare_dram_parameter("sink", [HD, SQ], BF16,
                                          isOutput=False)
    p["maskp"] = nc.declare_dram_parameter("maskp", [128, max(MW, 1)], BF16,
                                           isOutput=False)
    p["ones128h"] = nc.declare_dram_parameter("ones128h", [128, 128], BF16,
                                              isOutput=False)
    p["epsp"] = nc.declare_dram_parameter("epsp", [128, 1], F32,
                                          isOutput=False)
    p["oshard"] = nc.declare_dram_parameter("oshard", [D, KL], BF16,
                                            isOutput=True)

    with tile.TileContext(nc) as tc:
        _emit(nc, tc, p, klo_u, khi_max, moff)
    nc.finalize()
    return nc


def _emit(nc, tc, p, klo_u, khi_max, moff):
    pool = lambda name, bufs=1, space="SBUF": tc.tile_pool(
        name=name, bufs=bufs, space=space)
    mw = [max(0, khi_max[t] - klo_u[t]) for t in range(NT)]
    act_t = [t for t in range(NT) if klo_u[t] < KL]
    RG = [[0, 1, 2, 3], [4, 5, 6, 7]]

    with (
        pool("const") as constp,
        pool("persist") as persist,
        pool("wop") as wop,
        pool("dram", space="DRAM") as dramp,
    ):
        onesh_sb = constp.tile([128, 128], BF16, name="onesh_sb")
        eps_sb = constp.tile([128, 1], F32, name="eps_sb")
        cosq_sb = constp.tile([HD, KL], BF16, name="cosq_sb")
        sinq_sb = constp.tile([HD, KL], BF16, name="sinq_sb")
        cosk_sb = constp.tile([HD, SQ], BF16, name="cosk_sb")
        sink_sb = constp.tile([HD, SQ], BF16, name="sink_sb")
        mask_sb = constp.tile([128, max(int(moff[-1]), 1)], BF16,
                              name="mask_sb")

        # full-S k/v for all 4 kv heads (filled by AG readback); one tile
        # per head, cols [0:S] = kT, [S:2S] = v, so each (head, rank)
        # readback is a single DMA
        kv_sb = [persist.tile([128, 2 * S], BF16, tag=f"kv{h}", name=f"kv{h}")
                 for h in range(HKV)]
        kT_sb = [kv_sb[h][:, 0:S] for h in range(HKV)]
        v_sb = [kv_sb[h][:, S:2 * S] for h in range(HKV)]
        qT_sb = [persist.tile([HD, KL], BF16, tag=f"q{m}", name=f"q{m}")
                 for m in range(H)]
        outT_sb = [persist.tile([HD, KL], BF16, tag=f"o{m}", name=f"om{m}")
                   for m in range(H)]
        hq_sb = persist.tile([128, NDC * KL], BF16, name="hq_sb")
        # wq in its own pool, freed after phase Q so B's pools (incl the
        # o_proj SBUF accumulators) fit
        wqp = tc.tile_pool(name="wqp", bufs=1, space="SBUF")
        wqp_cm = wqp.__enter__()
        wq_sb = wqp_cm.tile([128, NDC * H * HD], BF16, name="wq_sb")

        # DRAM staging for the kv AllGathers (one per kv head, so head 0's
        # exchange completes as early as the CC stream allows and phase B's
        # head groups pipeline against the later gathers)
        kvloc = [dramp.tile([128, 2 * SQ], BF16, tag=f"kvl{i}",
                            name=f"kvl{i}") for i in range(HKV)]
        kvall = [dramp.tile([512, 2 * SQ], BF16, tag=f"kva{i}",
                            name=f"kva{i}") for i in range(HKV)]

        # ---------------- Phase A: k/v proj (s-quarter, 4 kv heads) ------
        with (
            pool("wkv") as wkvp,
            pool("ha") as hap,
            pool("sqa", bufs=2) as sqp,
            pool("rowa", bufs=2) as rowp,
            pool("kvout", bufs=4) as kvoutp,
            pool("psA", bufs=1, space="PSUM") as pA,
        ):
            wk_sb = wkvp.tile([128, NDC * HKV * HD], BF16, name="wk_sb")
            wv_sb = wkvp.tile([128, NDC * HKV * HD], BF16, name="wv_sb")
            ht = hap.tile([128, NDC * SQ], BF16, name="ht")
            nc.sync.dma_start(ht[:], p["hT"][:])
            nc.scalar.dma_start(wk_sb[:], p["wk"][:])
            nc.gpsimd.dma_start(wv_sb[:], p["wv"][:])
            # early consts ride the sync queue behind ht; Q-phase consts
            # behind wk on scalar; hq + wq on the vector queue
            nc.sync.dma_start(onesh_sb[:], p["ones128h"][:])
            nc.sync.dma_start(eps_sb[:], p["epsp"][:])
            nc.sync.dma_start(cosk_sb[:], p["cosk"][:])
            nc.sync.dma_start(sink_sb[:], p["sink"][:])
            nc.scalar.dma_start(cosq_sb[:], p["cosq"][:])
            nc.scalar.dma_start(sinq_sb[:], p["sinq"][:])
            nc.scalar.dma_start(mask_sb[:], p["maskp"][:])
            nc.sync.dma_start(hq_sb[:], p["hq"][:])
            # wq rides the scalar queue behind wk so the gpsimd queue only
            # carries wv + the collective triggers (a bulk SWDGE there
            # delays the AG triggers by its descriptor-generation time)
            nc.scalar.dma_start(wq_sb[:], p["wq"][:])

            for h in range(HKV):
                kraw = pA.tile([128, SQ], F32, tag="kraw", bufs=2,
                               name=f"kraw{h}")
                for dc in range(NDC):
                    nc.tensor.matmul(kraw[:],
                                     wk_sb[:, (dc * HKV + h) * HD:
                                           (dc * HKV + h + 1) * HD],
                                     ht[:, dc * SQ:(dc + 1) * SQ],
                                     start=(dc == 0), stop=(dc == NDC - 1))
                vp = pA.tile([128, SQ], F32, tag="vp", bufs=2, name=f"vp{h}")
                for dc in range(NDC):
                    for sub in range(4):
                        nc.tensor.matmul(
                            vp[:, sub * HD:(sub + 1) * HD],
                            ht[:, dc * SQ + sub * 128:dc * SQ + sub * 128
                               + 128],
                            wv_sb[:, (dc * HKV + h) * HD:
                                  (dc * HKV + h + 1) * HD],
                            start=(dc == 0 and sub == 0),
                            stop=(dc == NDC - 1), skip_group_check=True)
                # k-norm + rope chains; v copy on ACT
                sqk = sqp.tile([128, SQ], BF16, tag="sqk", name=f"sqk{h}")
                nc.scalar.square(sqk[:], kraw[:])
                kc = rowp.tile([128, SQ], F32, tag="kc", name=f"kc{h}")
                nc.vector.tensor_mul(kc[:], kraw[:], cosk_sb[:])
                ks = rowp.tile([128, SQ], F32, tag="ks", name=f"ks{h}")
                nc.vector.tensor_mul(ks[0:64, :], kraw[64:128, :],
                                     sink_sb[0:64, :])
                nc.vector.tensor_mul(ks[64:128, :], kraw[0:64, :],
                                     sink_sb[64:128, :])
                nc.vector.tensor_add(kc[:], kc[:], ks[:])
                msq = pA.tile([128, SQ], F32, tag="kraw", bufs=2,
                              name=f"msq{h}")
                nc.tensor.matmul(msq[:], onesh_sb[:], sqk[:],
                                 start=True, stop=True)
                t3 = rowp.tile([128, SQ], F32, tag="t3k", name=f"t3k{h}")
                nc.scalar.activation(t3[:], msq[:], AFT.Sqrt,
                                     bias=eps_sb[:], scale=1.0 / HD)
                comb = rowp.tile([128, SQ], F32, tag="combk", name=f"cbk{h}")
                nc.vector.reciprocal_approx_fast(comb[:], t3[:])
                ktq = kvoutp.tile([128, SQ], BF16, tag="ktq", name=f"ktq{h}")
                nc.vector.tensor_mul(ktq[:], kc[:], comb[:])
                vtq = kvoutp.tile([128, SQ], BF16, tag="vtq", name=f"vtq{h}")
                nc.scalar.activation(vtq[:], vp[:], AFT.Copy)
                nc.sync.dma_start(kvloc[h][:, 0:SQ], ktq[:])
                nc.sync.dma_start(kvloc[h][:, SQ:2 * SQ], vtq[:])
                nc.gpsimd.collective_compute(
                    "AllGather", mybir.AluOpType.bypass,
                    replica_groups=RG,
                    ins=[kvloc[h].opt()], outs=[kvall[h].opt()])
            # readback: head-major so head 0 is complete first; k+v of one
            # (head, rank) block in a single DMA via a 3-d dst pattern
            for h in range(HKV):
                for r in range(4):
                    dst = kv_sb[h].rearrange("p (half s) -> p half s",
                                             half=2)[:, :,
                                                     r * SQ:(r + 1) * SQ]
                    nc.sync.dma_start(
                        dst,
                        kvall[h][r * 128:(r + 1) * 128, :]
                        .rearrange("p (half s) -> p half s", half=2))

        # ---------------- Phase Q: q proj + q-norm + rope, 16 heads ------
        with (
            pool("sqb", bufs=2) as sqbp,
            pool("rowq", bufs=2) as rowqp,
            pool("pq", bufs=1, space="PSUM") as pq,
        ):
            for w in range(4):
                heads = range(4 * w, 4 * w + 4)
                qraw = {}
                for mp in (0, 1):
                    qraw2 = pq.tile([128, 512], F32, tag="qraw", bufs=4,
                                    name=f"qraw{w}_{mp}")
                    qraw[4 * w + 2 * mp] = qraw2[:, 0:KL]
                    qraw[4 * w + 2 * mp + 1] = qraw2[:, KL:512]
                # two heads share one PSUM bank: only the even head's first
                # matmul uses start (whole-bank pending-zero covers both)
                for dc in range(NDC):
                    for m in heads:
                        nc.tensor.matmul(
                            qraw[m],
                            wq_sb[:, (dc * H + m) * HD:(dc * H + m + 1) * HD],
                            hq_sb[:, dc * KL:(dc + 1) * KL],
                            start=(dc == 0 and m % 2 == 0),
                            stop=(dc == NDC - 1),
                            skip_group_check=True)
                msqq = {}
                for m in heads:
                    sqm = sqbp.tile([128, KL], BF16, tag="sqm",
                                    name=f"sqm{m}")
                    nc.scalar.square(sqm[:], qraw[m])
                    qc = rowqp.tile([128, KL], F32, tag="qc", name=f"qc{m}")
                    nc.vector.tensor_mul(qc[:], qraw[m], cosq_sb[:])
                    qs = rowqp.tile([128, KL], F32, tag="qs", name=f"qs{m}")
                    nc.vector.tensor_mul(qs[0:64, :], qraw[m][64:128, :],
                                         sinq_sb[0:64, :])
                    nc.vector.tensor_mul(qs[64:128, :], qraw[m][0:64, :],
                                         sinq_sb[64:128, :])
                    nc.vector.tensor_add(qc[:], qc[:], qs[:])
                    mq = pq.tile([128, KL], F32, tag="msqq", bufs=2,
                                 name=f"msqq{m}")
                    nc.tensor.matmul(mq[:], onesh_sb[:], sqm[:],
                                     start=True, stop=True)
                    msqq[m] = mq
                    t3q = rowqp.tile([128, KL], F32, tag="t3q",
                                     name=f"t3q{m}")
                    nc.scalar.activation(t3q[:], mq[:], AFT.Sqrt,
                                         bias=eps_sb[:], scale=1.0 / HD)
                    combq = rowqp.tile([128, KL], F32, tag="combq",
                                       name=f"combq{m}")
                    nc.vector.reciprocal_approx_fast(combq[:], t3q[:])
                    nc.vector.tensor_mul(qT_sb[m][:], qc[:], combq[:])
            # preload the exp table set while the PE winds down phase Q
            dummy = sqbp.tile([128, 1], F32, tag="dummy", name="dummy")
            nc.scalar.activation(dummy[:], eps_sb[:], AFT.Exp, scale=1.0)
        wqp.__exit__(None, None, None)

        # wo pool opens in wq's freed space; DMAs issue on the gpsimd
        # queue (idle after the AG triggers)
        wop2 = tc.tile_pool(name="wop2", bufs=1, space="SBUF")
        wop2_cm = wop2.__enter__()
        wo_sb = [wop2_cm.tile([128, D], BF16, tag=f"wo{m}", name=f"wo{m}")
                 for m in range(H)]
        for m in range(H):
            nc.gpsimd.dma_start(wo_sb[m][:],
                                p["wo"][m * 128:(m + 1) * 128, :])

        # ---------------- Phase B: attention, 16 heads -------------------
        # Software-pipelined by one head: PE runs head m+1's score matmuls
        # while ACT/DVE finish head m's exp/mask, then does head m's
        # rowsum+attn@v with et(m) already in SBUF -- the PE stream never
        # stalls on the exp, which keeps the p-state at full clock.
        packs = [tuple(act_t[i:i + 4]) for i in range(0, len(act_t), 4)]
        with (
            pool("expp", bufs=9) as expp,
            pool("rowb", bufs=2) as rowbp,
            pool("oacc") as oaccp,
            pool("oevict", bufs=4) as oev,
            pool("psc", bufs=2, space="PSUM") as psc,
            pool("pro", bufs=2, space="PSUM") as pro,
            pool("po", bufs=2, space="PSUM") as poolc,
        ):
            o_acc = [oaccp.tile([128, KL], BF16, tag=f"oacc{dc}",
                                name=f"oacc{dc}") for dc in range(NDC)]
            def emit_scores(m):
                h4 = m // 4
                ets = {}
                for pk in packs:
                    ws = [KL - klo_u[t] for t in pk]
                    # flush layout: t0,t1 end at col 512; t2,t3 start at 512
                    off = {}
                    n = len(pk)
                    if n >= 2:
                        off[pk[1]] = 512 - ws[1]
                        off[pk[0]] = 512 - ws[1] - ws[0]
                    else:
                        off[pk[0]] = 512 - ws[0]
                    c = 512
                    for i in range(2, n):
                        off[pk[i]] = c
                        c += ws[i]
                    base = off[pk[0]]
                    wtot = sum(ws)
                    sc = psc.tile([128, 1024], F32, tag="sc", name="sc")
                    # two tiles share each PSUM bank: start only on the
                    # bank's first tile (pending-zero covers the second)
                    for i, t in enumerate(pk):
                        nc.tensor.matmul(
                            sc[:, off[t]:off[t] + ws[i]],
                            kT_sb[h4][:, t * 128:(t + 1) * 128],
                            qT_sb[m][:, klo_u[t]:KL],
                            start=(i == 0 or i == 2), stop=True,
                            skip_group_check=True)
                    et = expp.tile([128, wtot], BF16, tag="et",
                                   name=f"et{m}_{pk[0]}")
                    nc.scalar.activation(et[:], sc[:, base:base + wtot],
                                         AFT.Exp, scale=SCALE)
                    for i, t in enumerate(pk):
                        ets[t] = (et, off[t] - base)
                        if mw[t] > 0:
                            eo = off[t] - base
                            nc.vector.tensor_mul(
                                et[:, eo:eo + mw[t]], et[:, eo:eo + mw[t]],
                                mask_sb[:, int(moff[t]):int(moff[t])
                                        + mw[t]])
                return ets

            def emit_reduce(m, ets):
                h4 = m // 4
                # rowsum in cols [0:KL], attn@v in cols [256:256+KL] of ONE
                # psum bank; outp's first matmul rides rowsum's whole-bank
                # pending-zero (start=False)
                ro = pro.tile([128, 512], F32, tag="ro", name="ro")
                first_t, last_t = act_t[0], act_t[-1]
                for t in act_t:
                    et, eo = ets[t]
                    nc.tensor.matmul(ro[:, klo_u[t]:KL], onesh_sb[:],
                                     et[:, eo:eo + KL - klo_u[t]],
                                     start=(t == first_t),
                                     stop=(t == last_t),
                                     skip_group_check=True)
                for t in act_t:
                    et, eo = ets[t]
                    nc.tensor.matmul(ro[:, 256 + klo_u[t]:256 + KL],
                                     v_sb[h4][:, t * 128:(t + 1) * 128],
                                     et[:, eo:eo + KL - klo_u[t]],
                                     start=False,
                                     stop=(t == last_t),
                                     skip_group_check=True)
                recip = rowbp.tile([128, KL], F32, tag="recip",
                                   name="recip")
                nc.vector.reciprocal_approx_fast(recip[:], ro[:, 0:KL])
                nc.vector.tensor_mul(outT_sb[m][:], ro[:, 256:256 + KL],
                                     recip[:])

            def emit_opass(g):
                # o_proj contribution of head-group g (heads 4g..4g+3),
                # accumulated into SBUF f32; the final group writes the
                # bf16 output tiles and DMAs them out. PE work slots into
                # the exp-bound stretches of phase B.
                for dc in range(NDC):
                    po = poolc.tile([128, 512], F32, tag="po", name="po")
                    for i in range(4):
                        m2 = 4 * g + i
                        nc.tensor.matmul(
                            po[:, 0:KL],
                            wo_sb[m2][:, dc * 128:(dc + 1) * 128],
                            outT_sb[m2][:],
                            start=(i == 0), stop=(i == 3))
                    if g == 0:
                        nc.vector.tensor_copy(o_acc[dc][:], po[:, 0:KL])
                    elif g < 3:
                        nc.vector.tensor_add(o_acc[dc][:], o_acc[dc][:],
                                             po[:, 0:KL])
                    else:
                        osb = oev.tile([128, KL], BF16, tag="osb",
                                       name="osb")
                        nc.vector.tensor_add(osb[:], o_acc[dc][:],
                                             po[:, 0:KL])
                        nc.sync.dma_start(
                            p["oshard"][dc * 128:(dc + 1) * 128, :], osb[:])

            prev = emit_scores(0)
            for m in range(H):
                cur = emit_scores(m + 1) if m + 1 < H else None
                emit_reduce(m, prev)
                prev = cur
                if m % 4 == 3:
                    emit_opass(m // 4)

        wop2.__exit__(None, None, None)


def kernel(hidden_states, pos_ids, cos, sin, w_ln, w_qn, w_kn,
           Wq, Wk, Wv, Wo, bo):
    h = np.asarray(hidden_states, dtype=np.float64)
    pos = np.asarray(pos_ids)
    cos0 = np.asarray(cos, dtype=np.float64)[0]          # [S, HD]
    sin0 = np.asarray(sin, dtype=np.float64)[0]
    w_ln = np.asarray(w_ln, dtype=np.float64)
    w_qn = np.asarray(w_qn, dtype=np.float64)
    w_kn = np.asarray(w_kn, dtype=np.float64)
    Wq = np.asarray(Wq, dtype=np.float64)
    Wk = np.asarray(Wk, dtype=np.float64)
    Wv = np.asarray(Wv, dtype=np.float64)
    Wo = np.asarray(Wo, dtype=np.float32)
    bo = np.asarray(bo, dtype=np.float32)

    order = np.argsort(pos, axis=1, kind="stable")
    pos_s = np.take_along_axis(pos, order, axis=1)       # sorted per batch

    klo = np.stack([np.searchsorted(pos_s[b], np.arange(NT + 1) * 128)
                    for b in range(B)])                   # [B, NT+1]
    # local (strided) bounds: query j on core g is global index 4j+g
    # klo_loc[t] = min over (b,g) of ceil((klo[b,t]-g)/4) = min_b klo//4,
    # 8-aligned down (PSUM matmul dst alignment)
    klo_loc = ((klo[:, :NT].min(axis=0) // 4) // 8 * 8).astype(int)
    # khi_loc[t] = max over (b,g) of ceil((khi[b,t]-g)/4) = max_b (khi+3)//4
    khi_loc = np.minimum((klo[:, 1:].max(axis=0) + 3) // 4, KL).astype(int)
    klo_u = klo_loc.tolist()
    khi_max = khi_loc.tolist()

    key = (tuple(klo_u), tuple(khi_max))
    if key not in _BUILD_CACHE:
        _BUILD_CACHE[key] = _build(klo_u, khi_max)
    nc = _BUILD_CACHE[key]

    Wq_f = w_ln[:, None] * Wq
    Wk_f = w_ln[:, None] * Wk
    Wv_f = w_ln[:, None] * Wv

    sgn = np.where(np.arange(HD) < 64, -1.0, 1.0)[:, None]
    wqn_sh = np.roll(w_qn, -64)[:, None]
    wkn_sh = np.roll(w_kn, -64)[:, None]

    mw = [max(0, khi_max[t] - klo_u[t]) for t in range(NT)]
    moff = np.concatenate([[0], np.cumsum(mw)]).astype(int)
    MW = max(int(moff[-1]), 1)

    wq_h = np.ascontiguousarray(
        Wq_f.reshape(NDC, 128, H, HD).transpose(1, 0, 2, 3)
        .reshape(128, NDC * H * HD).astype(BF))
    wk_h = np.ascontiguousarray(
        Wk_f.reshape(NDC, 128, HKV, HD).transpose(1, 0, 2, 3)
        .reshape(128, NDC * HKV * HD).astype(BF))
    wv_h = np.ascontiguousarray(
        Wv_f.reshape(NDC, 128, HKV, HD).transpose(1, 0, 2, 3)
        .reshape(128, NDC * HKV * HD).astype(BF))
    wo_h = np.ascontiguousarray(Wo.astype(BF))
    ones_h = np.ones((128, 128), dtype=BF)
    eps_h = np.full((128, 1), EPS, dtype=np.float32)

    p_arange = np.arange(128)[:, None]
    rs_all = 1.0 / np.sqrt((h ** 2).mean(axis=2) + EPS)   # [B, S] f64
    hn = h * rs_all[:, :, None]                           # pre-normed, f64
    in_maps = []
    for c in range(NCORES):
        b, g = c // 4, c % 4
        ps = pos_s[b]
        psl = ps[g::4]                                    # local 256 queries
        hTb = np.ascontiguousarray(
            hn[b][g * SQ:(g + 1) * SQ].T.reshape(NDC, 128, SQ)
            .transpose(1, 0, 2).reshape(128, NDC * SQ).astype(BF))
        hq_l = np.ascontiguousarray(
            hn[b][psl].T.reshape(NDC, 128, KL)
            .transpose(1, 0, 2).reshape(128, NDC * KL).astype(BF))
        COSQ = np.ascontiguousarray((w_qn[:, None] * cos0[psl].T).astype(BF))
        SINQ = np.ascontiguousarray(
            (wqn_sh * sin0[psl].T * sgn).astype(BF))
        ck = cos0[g * SQ:(g + 1) * SQ].T
        sk = sin0[g * SQ:(g + 1) * SQ].T
        COSK = np.ascontiguousarray((w_kn[:, None] * ck).astype(BF))
        SINK = np.ascontiguousarray((wkn_sh * sk * sgn).astype(BF))
        maskp = np.zeros((128, MW), dtype=BF)
        for t in range(NT):
            if mw[t] == 0:
                continue
            cols = psl[klo_u[t]:klo_u[t] + mw[t]][None, :]
            maskp[:, int(moff[t]):int(moff[t]) + mw[t]] = (
                (t * 128 + p_arange) <= cols).astype(BF)
        in_maps.append({
            "hT": hTb,
            "hq": hq_l,
            "wq": wq_h, "wk": wk_h, "wv": wv_h, "wo": wo_h,
            "cosq": COSQ, "sinq": SINQ, "cosk": COSK, "sink": SINK,
            "maskp": maskp,
            "ones128h": ones_h,
            "epsp": eps_h,
        })

    global _LAST_IN_MAPS
    _LAST_IN_MAPS = in_maps
    res = run_bass_kernel_spmd(nc, in_maps, list(range(NCORES)))

    out = np.zeros((B, S, D), dtype=np.float32)
    for c in range(NCORES):
        b, g = c // 4, c % 4
        psl = pos_s[b][g::4]
        oT = np.asarray(res.results[c]["oshard"]).astype(np.float32)
        out[b, psl, :] = oT.T + bo[None, :]
    return out
